# revision 4
# baseline (speedup 1.0000x reference)
"""Trainium2 Bass kernel for multi-head causal self-attention.

Tensor-parallel over 8 NeuronCores: each core owns 2 of the 16 heads.
Per core (SPMD, identical program, different weight shards):
  - QKV projections for its 2 heads (bf16 data, f32 psum accumulate)
  - causal attention for its 2 heads (scores kept transposed [k, q];
    softmax denominator via a ones-column fused into the PV matmul,
    PV streams N=129 in bf16)
  - output projection partial (f32r), inlined per q-block
Host: shards weights, pre-transposes X to bf16, sums the 8 partials,
adds bo (+ bv@Wo; bk drops out of softmax).

Engine split: tensor = all matmuls; scalar = exp only; vector = all
psum copy-outs, bias adds, reciprocals, mask multiplies.
"""
import numpy as np
from contextlib import ExitStack

import ml_dtypes
import concourse.bass as bass
import concourse.tile as tile
from concourse import bacc, mybir
from concourse.bass_utils import run_bass_kernel_spmd

# Problem shape (hardcoded per contract)
B, S, D = 2, 2048, 2048
H, DH = 16, 128
N_CORES = 8
HL = H // N_CORES          # heads per core = 2
DHL = HL * DH              # 256
SC = 256                   # s-chunk for projections
NSC = S // SC              # 8 chunks per batch
NKB = S // 128             # 16 key blocks per batch
NQI = S // 512             # 4 q-chunks of 512 per batch
NDC = D // 128             # 16 contraction blocks

F32 = mybir.dt.float32
F32R = mybir.dt.float32r
BF16 = mybir.dt.bfloat16
AF = mybir.ActivationFunctionType
BF16NP = ml_dtypes.bfloat16

_cached_nc = None


def _mm(nc, out, lhsT, rhs, start, stop):
    nc.tensor.matmul(out, lhsT, rhs, start=start, stop=stop)


def build_nc():
    nc = bacc.Bacc("TRN2", target_bir_lowering=False, debug=False, num_devices=N_CORES)

    xt = nc.dram_tensor("xt", [B, D, S], BF16, kind="ExternalInput").ap()
    wq = nc.dram_tensor("wq", [D, DHL], BF16, kind="ExternalInput").ap()
    wk = nc.dram_tensor("wk", [D, DHL], BF16, kind="ExternalInput").ap()
    wv = nc.dram_tensor("wv", [D, DHL], BF16, kind="ExternalInput").ap()
    bqt_d = nc.dram_tensor("bqt", [128, HL], F32, kind="ExternalInput").ap()
    wo = nc.dram_tensor("wo", [HL, 128, D], F32R, kind="ExternalInput").ap()
    mask_d = nc.dram_tensor("maskt", [128, 128], BF16, kind="ExternalInput").ap()
    vones_d = nc.dram_tensor("vones", [128, 1, 1, 1], BF16, kind="ExternalInput").ap()
    ident_d = nc.dram_tensor("ident", [128, 128], F32R, kind="ExternalInput").ap()
    out = nc.dram_tensor("out", [B, S, D], F32, kind="ExternalOutput").ap()

    wq_r = wq.rearrange("(n p) d -> p n d", p=128)
    wk_r = wk.rearrange("(n p) d -> p n d", p=128)
    wv_r = wv.rearrange("(n p) d -> p n d", p=128)

    with tile.TileContext(nc) as tc, ExitStack() as ctx:
        pp = ctx.enter_context(tc.tile_pool(name="persist", bufs=1))

        wq_t = pp.tile([128, NDC, DHL], BF16)
        wk_t = pp.tile([128, NDC, DHL], BF16)
        wv_t = pp.tile([128, NDC, DHL], BF16)
        wo_t = pp.tile([128, HL, D], F32R)
        bqt = pp.tile([128, HL], F32)
        mask = pp.tile([128, 128], BF16)
        ident = pp.tile([128, 128], F32R)

        nc.sync.dma_start(out=wq_t, in_=wq_r)
        nc.sync.dma_start(out=bqt, in_=bqt_d)

        xp = ctx.enter_context(tc.tile_pool(name="xtp", bufs=3))

        for b in range(B):
            with tc.tile_pool(name=f"bat{b}", bufs=1) as bp:
                qt = bp.tile([128, HL, S], BF16)   # Q^T per head
                kt = bp.tile([128, HL, S], BF16)   # K^T per head
                # per (key block, head): [V_h (128) | ones | pad]
                vcat = bp.tile([128, NKB, HL, 130], BF16)
                for h in range(HL):
                    nc.sync.dma_start(
                        out=vcat[:, :, h, 128:129],
                        in_=vones_d[:, :, 0].to_broadcast([128, NKB, 1]))

                xt_r = xt[b].rearrange("(n p) s -> p n s", p=128)

                # ---- QKV projections ----
                with tc.tile_pool(name=f"psp{b}", bufs=1, space="PSUM") as psp:
                    for sc in range(NSC):
                        xt_t = xp.tile([128, NDC, SC], BF16, tag="xt")
                        nc.gpsimd.dma_start(
                            out=xt_t, in_=xt_r[:, :, sc * SC:(sc + 1) * SC]
                        )
                        if b == 0 and sc == 0:
                            # stream the rest of the weights behind x chunk 0
                            nc.sync.dma_start(out=wk_t, in_=wk_r)
                            nc.sync.dma_start(out=wv_t, in_=wv_r)
                        for h in range(HL):
                            psq = psp.tile([128, SC], F32, tag="pq", bufs=1)
                            psk = psp.tile([128, SC], F32, tag="pk", bufs=1)
                            for dc in range(NDC):
                                _mm(nc, psq, wq_t[:, dc, h * 128:(h + 1) * 128],
                                    xt_t[:, dc, :], dc == 0, dc == NDC - 1)
                            nc.vector.tensor_scalar_add(
                                out=qt[:, h, sc * SC:(sc + 1) * SC], in0=psq,
                                scalar1=bqt[:, h:h + 1])
                            for dc in range(NDC):
                                _mm(nc, psk, wk_t[:, dc, h * 128:(h + 1) * 128],
                                    xt_t[:, dc, :], dc == 0, dc == NDC - 1)
                            nc.vector.tensor_copy(
                                kt[:, h, sc * SC:(sc + 1) * SC], psk)
                        for sb in range(SC // 128):
                            kb = sc * (SC // 128) + sb
                            psv = psp.tile([128, DHL], F32, tag="pv", bufs=1)
                            for dc in range(NDC):
                                _mm(nc, psv, xt_t[:, dc, sb * 128:(sb + 1) * 128],
                                    wv_t[:, dc, :], dc == 0, dc == NDC - 1)
                            nc.vector.tensor_copy(
                                vcat[:, kb, 0, 0:128], psv[:, 0:128])
                            nc.vector.tensor_copy(
                                vcat[:, kb, 1, 0:128], psv[:, 128:256])

                if b == 0:
                    nc.sync.dma_start(out=wo_t[:, 0, :], in_=wo[0])
                    nc.sync.dma_start(out=wo_t[:, 1, :], in_=wo[1])
                    nc.sync.dma_start(out=mask, in_=mask_d)
                    nc.sync.dma_start(out=ident, in_=ident_d)

                # ---- causal attention + inline output projection ----
                with tc.tile_pool(name=f"ex{b}", bufs=3) as xpool, \
                     tc.tile_pool(name=f"sm{b}", bufs=2) as sm, \
                     tc.tile_pool(name=f"psa{b}", bufs=1, space="PSUM") as psa:
                    for qi in range(NQI):
                        st = {}
                        for h in range(HL):
                            accs = [psa.tile([128, 256], F32, tag=f"acc{i}",
                                             bufs=1, name=f"acc{i}")
                                    for i in range(4)]
                            nkb = 4 * qi + 4
                            # scores pipelined one kb ahead of exp/PV
                            pss_t = {}
                            exs = {}

                            def score(kb):
                                dq = max(0, (kb - 4 * qi)) * 128
                                pss = psa.tile([128, 512], F32, tag="sc",
                                               bufs=2)
                                _mm(nc, pss[:, dq:512],
                                    kt[:, h, kb * 128:(kb + 1) * 128],
                                    qt[:, h, qi * 512 + dq:(qi + 1) * 512],
                                    True, True)
                                pss_t[kb] = pss

                            score(0)
                            for kb in range(nkb):
                                dq = max(0, (kb - 4 * qi)) * 128
                                pss = pss_t.pop(kb)
                                ex = xpool.tile([128, 512], BF16, tag="ex",
                                                bufs=6)
                                nc.scalar.activation(
                                    out=ex[:, dq:512], in_=pss[:, dq:512],
                                    func=AF.Exp)
                                if kb + 1 < nkb:
                                    score(kb + 1)
                                if kb >= 4 * qi:
                                    nc.vector.tensor_mul(
                                        ex[:, dq:dq + 128],
                                        ex[:, dq:dq + 128], mask)
                                for qql in range(4):
                                    qq = 4 * qi + qql
                                    if kb <= qq:
                                        _mm(nc, accs[qql][:, 0:129],
                                            ex[:, qql * 128:(qql + 1) * 128],
                                            vcat[:, kb, h, 0:129],
                                            kb == 0, kb == qq)
                            for qql in range(4):
                                rc = sm.tile([128, 1], F32, tag="rc")
                                nc.vector.reciprocal(rc, accs[qql][:, 128:129])
                                an = sm.tile([128, 128], F32R, tag="an")
                                nc.vector.tensor_scalar_mul(
                                    out=an, in0=accs[qql][:, 0:128],
                                    scalar1=rc)
                                pst = psa.tile([128, 128], F32R, tag="sc",
                                               bufs=2)
                                nc.tensor.transpose(pst, an, ident)
                                stt = sm.tile([128, 128], F32R, tag="stt",
                                              bufs=10)
                                nc.vector.tensor_copy(stt, pst)
                                st[(h, qql)] = stt
                        # output projection for these 4 q-blocks
                        for qql in range(4):
                            qq = 4 * qi + qql
                            for dk in range(D // 512):
                                po = psa.tile([128, 512], F32, tag="po", bufs=2)
                                _mm(nc, po, st[(0, qql)],
                                    wo_t[:, 0, dk * 512:(dk + 1) * 512],
                                    True, False)
                                _mm(nc, po, st[(1, qql)],
                                    wo_t[:, 1, dk * 512:(dk + 1) * 512],
                                    False, True)
                                ot = sm.tile([128, 512], F32, tag="ot", bufs=4)
                                nc.vector.tensor_copy(ot, po)
                                nc.sync.dma_start(
                                    out=out[b, qq * 128:(qq + 1) * 128,
                                            dk * 512:(dk + 1) * 512],
                                    in_=ot)

    nc.compile()
    return nc


def _get_nc():
    global _cached_nc
    if _cached_nc is None:
        _cached_nc = build_nc()
    return _cached_nc


def make_in_maps(X, Wq, bq, Wk, bk, Wv, bv, Wo, bo):
    X = np.asarray(X, dtype=np.float32)
    scale = np.float32(1.0 / np.sqrt(DH))
    XT = np.ascontiguousarray(X.transpose(0, 2, 1)).astype(BF16NP)
    mask = (np.arange(128)[None, :] >= np.arange(128)[:, None])
    mask = mask.astype(BF16NP)
    ident = np.eye(128, dtype=np.float32)
    in_maps = []
    for c in range(N_CORES):
        hs = slice(c * DHL, (c + 1) * DHL)
        in_maps.append({
            "xt": XT,
            "wq": np.ascontiguousarray(
                np.asarray(Wq, np.float32)[:, hs] * scale).astype(BF16NP),
            "wk": np.ascontiguousarray(
                np.asarray(Wk, np.float32)[:, hs]).astype(BF16NP),
            "wv": np.ascontiguousarray(
                np.asarray(Wv, np.float32)[:, hs]).astype(BF16NP),
            "bqt": np.ascontiguousarray(
                (np.asarray(bq, np.float32)[hs] * scale).reshape(HL, 128).T),
            "wo": np.ascontiguousarray(
                np.asarray(Wo, np.float32)[hs, :].reshape(HL, 128, D)),
            "maskt": mask,
            "ident": ident,
            "vones": np.ones((128, 1, 1, 1), BF16NP),
        })
    return in_maps


def kernel(X, Wq, bq, Wk, bk, Wv, bv, Wo, bo, _trace=False):
    nc = _get_nc()
    in_maps = make_in_maps(X, Wq, bq, Wk, bk, Wv, bv, Wo, bo)
    res = run_bass_kernel_spmd(nc, in_maps, list(range(N_CORES)), trace=_trace)
    acc = res.results[0]["out"].astype(np.float64)
    for c in range(1, N_CORES):
        acc += res.results[c]["out"]
    # bv commutes through softmax: sum_k w_k (v_k + bv) = (sum_k w_k v_k) + bv,
    # so the V bias contributes bv @ Wo, folded here with bo. bk shifts every
    # score for a given query by the same constant, so it cancels in softmax.
    acc += np.asarray(bo, np.float64) + (
        np.asarray(bv, np.float64) @ np.asarray(Wo, np.float64))
    out = acc.astype(np.float32)
    if _trace:
        return out, res
    return out


# revision 6
# speedup vs baseline: 1.3283x; 1.3283x over previous
"""Trainium2 Bass kernel for multi-head causal self-attention.

Tensor-parallel over 8 NeuronCores: each core owns 2 of the 16 heads.
Per core (SPMD, identical program, different weight shards):
  - QKV projections for its 2 heads (bf16 data, f32 psum accumulate)
  - causal attention for its 2 heads (scores kept transposed [k, q];
    softmax denominator via a ones-column fused into the PV matmul,
    PV streams N=129 in bf16)
  - output projection partial (f32r), inlined per q-block
Host: shards weights, pre-transposes X to bf16, sums the 8 partials,
adds bo (+ bv@Wo; bk drops out of softmax).

Engine split: tensor = all matmuls; scalar = exp only; vector = all
psum copy-outs, bias adds, reciprocals, mask multiplies.
"""
import numpy as np
from contextlib import ExitStack

import ml_dtypes
import concourse.bass as bass
import concourse.tile as tile
from concourse import bacc, mybir
from concourse.bass_utils import run_bass_kernel_spmd

# Problem shape (hardcoded per contract)
B, S, D = 2, 2048, 2048
H, DH = 16, 128
N_CORES = 8
HL = H // N_CORES          # heads per core = 2
DHL = HL * DH              # 256
SC = 256                   # s-chunk for projections
NSC = S // SC              # 8 chunks per batch
NKB = S // 128             # 16 key blocks per batch
NQI = S // 512             # 4 q-chunks of 512 per batch
NDC = D // 128             # 16 contraction blocks

F32 = mybir.dt.float32
F32R = mybir.dt.float32r
BF16 = mybir.dt.bfloat16
AF = mybir.ActivationFunctionType
BF16NP = ml_dtypes.bfloat16

_cached_nc = None


def _mm(nc, out, lhsT, rhs, start, stop):
    nc.tensor.matmul(out, lhsT, rhs, start=start, stop=stop)


def build_nc():
    nc = bacc.Bacc("TRN2", target_bir_lowering=False, debug=False, num_devices=N_CORES)

    xt = nc.dram_tensor("xt", [B, D, S], BF16, kind="ExternalInput").ap()
    wq = nc.dram_tensor("wq", [D, DHL], BF16, kind="ExternalInput").ap()
    wk = nc.dram_tensor("wk", [D, DHL], BF16, kind="ExternalInput").ap()
    wv = nc.dram_tensor("wv", [D, DHL], BF16, kind="ExternalInput").ap()
    bqt_d = nc.dram_tensor("bqt", [128, HL], F32, kind="ExternalInput").ap()
    wo = nc.dram_tensor("wo", [HL, 128, D], F32R, kind="ExternalInput").ap()
    mask_d = nc.dram_tensor("maskt", [128, 128], BF16, kind="ExternalInput").ap()
    vones_d = nc.dram_tensor("vones", [128, 1, 1, 1], BF16, kind="ExternalInput").ap()
    ident_d = nc.dram_tensor("ident", [128, 128], F32R, kind="ExternalInput").ap()
    out = nc.dram_tensor("out", [B, S, D], F32, kind="ExternalOutput").ap()

    wq_r = wq.rearrange("(n p) d -> p n d", p=128)
    wk_r = wk.rearrange("(n p) d -> p n d", p=128)
    wv_r = wv.rearrange("(n p) d -> p n d", p=128)

    with tile.TileContext(nc) as tc, ExitStack() as ctx:
        pp = ctx.enter_context(tc.tile_pool(name="persist", bufs=1))

        wq_t = pp.tile([128, NDC, DHL], BF16)
        wk_t = pp.tile([128, NDC, DHL], BF16)
        wv_t = pp.tile([128, NDC, DHL], BF16)
        wo_t = pp.tile([128, HL, D], F32R)
        bqt = pp.tile([128, HL], F32)
        mask = pp.tile([128, 128], BF16)
        ident = pp.tile([128, 128], F32R)

        nc.sync.dma_start(out=wq_t, in_=wq_r)
        nc.sync.dma_start(out=bqt, in_=bqt_d)

        xp = ctx.enter_context(tc.tile_pool(name="xtp", bufs=3))

        for b in range(B):
            with tc.tile_pool(name=f"bat{b}", bufs=1) as bp:
                qt = bp.tile([128, HL, S], BF16)   # Q^T per head
                kt = bp.tile([128, HL, S], BF16)   # K^T per head
                # per (key block, head): [V_h (128) | ones | pad]
                vcat = bp.tile([128, NKB, HL, 130], BF16)
                nc.vector.memset(vcat[:, :, :, 128:129], 1.0)

                xt_r = xt[b].rearrange("(n p) s -> p n s", p=128)

                # ---- QKV projections ----
                with tc.tile_pool(name=f"psp{b}", bufs=1, space="PSUM") as psp:
                    for sc in range(NSC):
                        xt_t = xp.tile([128, NDC, SC], BF16, tag="xt")
                        nc.gpsimd.dma_start(
                            out=xt_t, in_=xt_r[:, :, sc * SC:(sc + 1) * SC]
                        )
                        if b == 0 and sc == 0:
                            # stream the rest of the weights behind x chunk 0
                            nc.sync.dma_start(out=wk_t, in_=wk_r)
                            nc.sync.dma_start(out=wv_t, in_=wv_r)
                        for h in range(HL):
                            psq = psp.tile([128, SC], F32, tag="pq", bufs=1)
                            psk = psp.tile([128, SC], F32, tag="pk", bufs=1)
                            for dc in range(NDC):
                                _mm(nc, psq, wq_t[:, dc, h * 128:(h + 1) * 128],
                                    xt_t[:, dc, :], dc == 0, dc == NDC - 1)
                            nc.vector.tensor_scalar_add(
                                out=qt[:, h, sc * SC:(sc + 1) * SC], in0=psq,
                                scalar1=bqt[:, h:h + 1])
                            for dc in range(NDC):
                                _mm(nc, psk, wk_t[:, dc, h * 128:(h + 1) * 128],
                                    xt_t[:, dc, :], dc == 0, dc == NDC - 1)
                            nc.vector.tensor_copy(
                                kt[:, h, sc * SC:(sc + 1) * SC], psk)
                        for sb in range(SC // 128):
                            kb = sc * (SC // 128) + sb
                            psv = psp.tile([128, DHL], F32, tag="pv", bufs=2)
                            for dc in range(NDC):
                                _mm(nc, psv, xt_t[:, dc, sb * 128:(sb + 1) * 128],
                                    wv_t[:, dc, :], dc == 0, dc == NDC - 1)
                            nc.vector.tensor_copy(
                                vcat[:, kb, 0, 0:128], psv[:, 0:128])
                            nc.vector.tensor_copy(
                                vcat[:, kb, 1, 0:128], psv[:, 128:256])

                if b == 0:
                    nc.sync.dma_start(out=wo_t[:, 0, :], in_=wo[0])
                    nc.sync.dma_start(out=wo_t[:, 1, :], in_=wo[1])
                    nc.sync.dma_start(out=mask, in_=mask_d)
                    nc.sync.dma_start(out=ident, in_=ident_d)

                # ---- causal attention + inline output projection ----
                with tc.tile_pool(name=f"ex{b}", bufs=3) as xpool, \
                     tc.tile_pool(name=f"sm{b}", bufs=2) as sm, \
                     tc.tile_pool(name=f"psa{b}", bufs=1, space="PSUM") as psa:
                    for qi in range(NQI):
                        st = {}
                        for h in range(HL):
                            accs = [psa.tile([128, 256], F32, tag=f"acc{i}",
                                             bufs=1, name=f"acc{i}")
                                    for i in range(4)]
                            nkb = 4 * qi + 4
                            # scores pipelined one kb ahead of exp/PV
                            pss_t = {}
                            exs = {}

                            def score(kb):
                                dq = max(0, (kb - 4 * qi)) * 128
                                pss = psa.tile([128, 512], F32, tag="sc",
                                               bufs=2)
                                _mm(nc, pss[:, dq:512],
                                    kt[:, h, kb * 128:(kb + 1) * 128],
                                    qt[:, h, qi * 512 + dq:(qi + 1) * 512],
                                    True, True)
                                pss_t[kb] = pss

                            score(0)
                            for kb in range(nkb):
                                dq = max(0, (kb - 4 * qi)) * 128
                                pss = pss_t.pop(kb)
                                ex = xpool.tile([128, 512], BF16, tag="ex",
                                                bufs=6)
                                nc.scalar.activation(
                                    out=ex[:, dq:512], in_=pss[:, dq:512],
                                    func=AF.Exp)
                                if kb + 1 < nkb:
                                    score(kb + 1)
                                if kb >= 4 * qi:
                                    nc.vector.tensor_mul(
                                        ex[:, dq:dq + 128],
                                        ex[:, dq:dq + 128], mask)
                                for qql in range(4):
                                    qq = 4 * qi + qql
                                    if kb <= qq:
                                        _mm(nc, accs[qql][:, 0:129],
                                            ex[:, qql * 128:(qql + 1) * 128],
                                            vcat[:, kb, h, 0:129],
                                            kb == 0, kb == qq)
                            for qql in range(4):
                                rc = sm.tile([128, 1], F32, tag="rc")
                                nc.vector.reciprocal(rc, accs[qql][:, 128:129])
                                an = sm.tile([128, 128], F32R, tag="an")
                                nc.vector.tensor_scalar_mul(
                                    out=an, in0=accs[qql][:, 0:128],
                                    scalar1=rc)
                                pst = psa.tile([128, 128], F32R, tag="sc",
                                               bufs=2)
                                nc.tensor.transpose(pst, an, ident)
                                stt = sm.tile([128, 128], F32R, tag="stt",
                                              bufs=10)
                                nc.vector.tensor_copy(stt, pst)
                                st[(h, qql)] = stt
                        # output projection for these 4 q-blocks
                        for qql in range(4):
                            qq = 4 * qi + qql
                            for dk in range(D // 512):
                                po = psa.tile([128, 512], F32, tag="po", bufs=2)
                                _mm(nc, po, st[(0, qql)],
                                    wo_t[:, 0, dk * 512:(dk + 1) * 512],
                                    True, False)
                                _mm(nc, po, st[(1, qql)],
                                    wo_t[:, 1, dk * 512:(dk + 1) * 512],
                                    False, True)
                                ot = sm.tile([128, 512], F32, tag="ot", bufs=4)
                                nc.vector.tensor_copy(ot, po)
                                nc.sync.dma_start(
                                    out=out[b, qq * 128:(qq + 1) * 128,
                                            dk * 512:(dk + 1) * 512],
                                    in_=ot)

    nc.compile()
    return nc


def _get_nc():
    global _cached_nc
    if _cached_nc is None:
        _cached_nc = build_nc()
    return _cached_nc


def make_in_maps(X, Wq, bq, Wk, bk, Wv, bv, Wo, bo):
    X = np.asarray(X, dtype=np.float32)
    scale = np.float32(1.0 / np.sqrt(DH))
    XT = np.ascontiguousarray(X.transpose(0, 2, 1)).astype(BF16NP)
    mask = (np.arange(128)[None, :] >= np.arange(128)[:, None])
    mask = mask.astype(BF16NP)
    ident = np.eye(128, dtype=np.float32)
    in_maps = []
    for c in range(N_CORES):
        hs = slice(c * DHL, (c + 1) * DHL)
        in_maps.append({
            "xt": XT,
            "wq": np.ascontiguousarray(
                np.asarray(Wq, np.float32)[:, hs] * scale).astype(BF16NP),
            "wk": np.ascontiguousarray(
                np.asarray(Wk, np.float32)[:, hs]).astype(BF16NP),
            "wv": np.ascontiguousarray(
                np.asarray(Wv, np.float32)[:, hs]).astype(BF16NP),
            "bqt": np.ascontiguousarray(
                (np.asarray(bq, np.float32)[hs] * scale).reshape(HL, 128).T),
            "wo": np.ascontiguousarray(
                np.asarray(Wo, np.float32)[hs, :].reshape(HL, 128, D)),
            "maskt": mask,
            "ident": ident,
            "vones": np.ones((128, 1, 1, 1), BF16NP),
        })
    return in_maps


def kernel(X, Wq, bq, Wk, bk, Wv, bv, Wo, bo, _trace=False):
    nc = _get_nc()
    in_maps = make_in_maps(X, Wq, bq, Wk, bk, Wv, bv, Wo, bo)
    res = run_bass_kernel_spmd(nc, in_maps, list(range(N_CORES)), trace=_trace)
    acc = res.results[0]["out"].astype(np.float64)
    for c in range(1, N_CORES):
        acc += res.results[c]["out"]
    # bv commutes through softmax: sum_k w_k (v_k + bv) = (sum_k w_k v_k) + bv,
    # so the V bias contributes bv @ Wo, folded here with bo. bk shifts every
    # score for a given query by the same constant, so it cancels in softmax.
    acc += np.asarray(bo, np.float64) + (
        np.asarray(bv, np.float64) @ np.asarray(Wo, np.float64))
    out = acc.astype(np.float32)
    if _trace:
        return out, res
    return out


# revision 9
# speedup vs baseline: 1.5618x; 1.1758x over previous
"""Trainium2 Bass kernel for multi-head causal self-attention.

Tensor-parallel over 8 NeuronCores: each core owns 2 of the 16 heads.
Per core (SPMD, identical program, different weight shards):
  - QKV projections for its 2 heads (bf16 data, f32 psum accumulate)
  - causal attention for its 2 heads (scores kept transposed [k, q];
    softmax denominator via a ones-column fused into the PV matmul,
    PV streams N=129 in bf16)
  - output projection partial (f32r), deferred and woven into the next
    phase so the tensor engine never starves behind scalar-engine exps
Phase schedule (tensor-heavy QKV woven into scalar-heavy attention):
  QKV(b0) -> att(b0) + QKV(b1) woven -> att(b1) + outproj(b0) woven
  -> outproj(b1) tail.
Host: shards weights, pre-transposes X to bf16, sums the 8 partials,
adds bo (+ bv@Wo; bk drops out of softmax).

Engine split: tensor = all matmuls; scalar = exp only; vector = psum
copy-outs, bias adds, reciprocals, mask multiplies, ones memset.
"""
import numpy as np
from contextlib import ExitStack

import ml_dtypes
import concourse.bass as bass
import concourse.tile as tile
from concourse import bacc, mybir
from concourse.bass_utils import run_bass_kernel_spmd

# Problem shape (hardcoded per contract)
B, S, D = 2, 2048, 2048
H, DH = 16, 128
N_CORES = 8
HL = H // N_CORES          # heads per core = 2
DHL = HL * DH              # 256
SC = 256                   # s-chunk for projections
NSC = S // SC              # 8 chunks per batch
NKB = S // 128             # 16 key blocks per batch
NQI = S // 512             # 4 q-chunks of 512 per batch
NDC = D // 128             # 16 contraction blocks

F32 = mybir.dt.float32
F32R = mybir.dt.float32r
BF16 = mybir.dt.bfloat16
AF = mybir.ActivationFunctionType
BF16NP = ml_dtypes.bfloat16

_cached_nc = None


def _mm(nc, out, lhsT, rhs, start, stop):
    nc.tensor.matmul(out, lhsT, rhs, start=start, stop=stop)


def _drain(gen):
    if gen is not None:
        for _ in gen:
            pass


class _Pacer:
    """Spread `total` weave quanta over `slots` call sites."""

    def __init__(self, gen, total, slots):
        self.gen = gen
        self.total = total
        self.slots = max(slots, 1)
        self.emitted = 0
        self.slot = 0

    def step(self):
        self.slot += 1
        while (self.gen is not None
               and self.emitted * self.slots < self.slot * self.total):
            try:
                next(self.gen)
                self.emitted += 1
            except StopIteration:
                self.gen = None

    def drain(self):
        _drain(self.gen)
        self.gen = None


def build_nc():
    nc = bacc.Bacc("TRN2", target_bir_lowering=False, debug=False, num_devices=N_CORES)

    xt = nc.dram_tensor("xt", [B, D, S], BF16, kind="ExternalInput").ap()
    wq = nc.dram_tensor("wq", [D, DHL], BF16, kind="ExternalInput").ap()
    wk = nc.dram_tensor("wk", [D, DHL], BF16, kind="ExternalInput").ap()
    wv = nc.dram_tensor("wv", [D, DHL], BF16, kind="ExternalInput").ap()
    bqt_d = nc.dram_tensor("bqt", [128, HL], F32, kind="ExternalInput").ap()
    wo = nc.dram_tensor("wo", [HL, 128, D], F32R, kind="ExternalInput").ap()
    mask_d = nc.dram_tensor("maskt", [128, 128], BF16, kind="ExternalInput").ap()
    ident_d = nc.dram_tensor("ident", [128, 128], F32R, kind="ExternalInput").ap()
    out = nc.dram_tensor("out", [B, S, D], BF16, kind="ExternalOutput").ap()

    wq_r = wq.rearrange("(n p) d -> p n d", p=128)
    wk_r = wk.rearrange("(n p) d -> p n d", p=128)
    wv_r = wv.rearrange("(n p) d -> p n d", p=128)

    with tile.TileContext(nc) as tc, ExitStack() as ctx:
        pp = ctx.enter_context(tc.tile_pool(name="persist", bufs=1))

        wq_t = pp.tile([128, NDC, DHL], BF16)
        wk_t = pp.tile([128, NDC, DHL], BF16)
        wv_t = pp.tile([128, NDC, DHL], BF16)
        wo_t = pp.tile([128, HL, D], F32R)
        bqt = pp.tile([128, HL], F32)
        mask = pp.tile([128, 128], BF16)
        ident = pp.tile([128, 128], F32R)

        nc.sync.dma_start(out=wq_t, in_=wq_r)
        nc.sync.dma_start(out=bqt, in_=bqt_d)

        # per-batch persistent activations
        qt = [pp.tile([128, HL, S], BF16, name=f"qt{b}") for b in range(B)]
        kt = [pp.tile([128, HL, S], BF16, name=f"kt{b}") for b in range(B)]
        vcat = [pp.tile([128, NKB, HL, 130], BF16, name=f"vcat{b}")
                for b in range(B)]
        # attn^T blocks awaiting (deferred) output projection
        stb = [pp.tile([128, NQI, 4, HL, 128], F32R, name=f"stb{b}")
               for b in range(B)]

        xp = ctx.enter_context(tc.tile_pool(name="xtp", bufs=3))
        sm = ctx.enter_context(tc.tile_pool(name="smp", bufs=2))
        xpool = ctx.enter_context(tc.tile_pool(name="expool", bufs=2))

        def qkv_quanta(b):
            """Generator: QKV projections for batch b, yielding after each
            16-matmul accumulation group (the weave quantum)."""
            with tc.tile_pool(name=f"psp{b}", bufs=1, space="PSUM") as psp:
                nc.vector.memset(vcat[b][:, :, :, 128:129], 1.0)
                xt_r = xt[b].rearrange("(n p) s -> p n s", p=128)
                for sc in range(NSC):
                    xt_t = xp.tile([128, NDC, SC], BF16, tag="xt")
                    nc.gpsimd.dma_start(
                        out=xt_t, in_=xt_r[:, :, sc * SC:(sc + 1) * SC]
                    )
                    if b == 0 and sc == 0:
                        nc.sync.dma_start(out=wk_t, in_=wk_r)
                        nc.sync.dma_start(out=wv_t, in_=wv_r)
                    for h in range(HL):
                        # q and k share one bank: their accumulation groups
                        # are strictly sequential (q closes before k opens)
                        pqk = psp.tile([128, 2, SC], F32, tag="pqk", bufs=1)
                        for dc in range(NDC):
                            _mm(nc, pqk[:, 0, :], wq_t[:, dc, h * 128:(h + 1) * 128],
                                xt_t[:, dc, :], dc == 0, dc == NDC - 1)
                        nc.vector.tensor_scalar_add(
                            out=qt[b][:, h, sc * SC:(sc + 1) * SC],
                            in0=pqk[:, 0, :], scalar1=bqt[:, h:h + 1])
                        yield
                        for dc in range(NDC):
                            _mm(nc, pqk[:, 1, :], wk_t[:, dc, h * 128:(h + 1) * 128],
                                xt_t[:, dc, :], dc == 0, dc == NDC - 1)
                        nc.vector.tensor_copy(
                            kt[b][:, h, sc * SC:(sc + 1) * SC], pqk[:, 1, :])
                        yield
                    psv = psp.tile([128, 2, DHL], F32, tag="pv", bufs=1)
                    for sb in range(SC // 128):
                        kb = sc * (SC // 128) + sb
                        pv = psv[:, sb, :]
                        for dc in range(NDC):
                            _mm(nc, pv, xt_t[:, dc, sb * 128:(sb + 1) * 128],
                                wv_t[:, dc, :], dc == 0, dc == NDC - 1)
                        nc.vector.tensor_copy(
                            vcat[b][:, kb, 0, 0:128], pv[:, 0:128])
                        nc.vector.tensor_copy(
                            vcat[b][:, kb, 1, 0:128], pv[:, 128:256])
                        yield
                if b == 0:
                    nc.sync.dma_start(out=wo_t[:, 0, :], in_=wo[0])
                    nc.sync.dma_start(out=wo_t[:, 1, :], in_=wo[1])
                    nc.sync.dma_start(out=mask, in_=mask_d)
                    nc.sync.dma_start(out=ident, in_=ident_d)

        def outproj_quanta(b, psa):
            """Generator: output projection for batch b from stb[b], using
            the po tag of the given PSUM pool. Yields per 512-col chunk."""
            for qi in range(NQI):
                for qql in range(4):
                    qq = 4 * qi + qql
                    for dk in range(D // 512):
                        po = psa.tile([128, 512], F32, tag="po", bufs=2)
                        _mm(nc, po, stb[b][:, qi, qql, 0, :],
                            wo_t[:, 0, dk * 512:(dk + 1) * 512], True, False)
                        _mm(nc, po, stb[b][:, qi, qql, 1, :],
                            wo_t[:, 1, dk * 512:(dk + 1) * 512], False, True)
                        ot = sm.tile([128, 512], BF16, tag="ot", bufs=4)
                        nc.vector.tensor_copy(ot, po)
                        nc.sync.dma_start(
                            out=out[b, qq * 128:(qq + 1) * 128,
                                    dk * 512:(dk + 1) * 512],
                            in_=ot)
                        yield

        def attention(b, psa, weave):
            """Causal attention for batch b; calls weave.step() at each
            opportunity to inject foreign tensor-engine work."""
            for qi in range(NQI):
                for h in range(HL):
                    accs = [psa.tile([128, 256], F32, tag=f"acc{i}",
                                     bufs=1, name=f"acc{i}")
                            for i in range(4)]
                    nkb = 4 * qi + 4
                    pss_t = {}

                    def score(kb):
                        dq = max(0, (kb - 4 * qi)) * 128
                        pss = psa.tile([128, 512], F32, tag="sc", bufs=2)
                        _mm(nc, pss[:, dq:512],
                            kt[b][:, h, kb * 128:(kb + 1) * 128],
                            qt[b][:, h, qi * 512 + dq:(qi + 1) * 512],
                            True, True)
                        pss_t[kb] = pss

                    score(0)
                    for kb in range(nkb):
                        dq = max(0, (kb - 4 * qi)) * 128
                        pss = pss_t.pop(kb)
                        ex = xpool.tile([128, 512], BF16, tag="ex", bufs=6)
                        nc.scalar.activation(
                            out=ex[:, dq:512], in_=pss[:, dq:512],
                            func=AF.Exp)
                        if kb + 1 < nkb:
                            score(kb + 1)
                        if kb >= 4 * qi:
                            nc.vector.tensor_mul(
                                ex[:, dq:dq + 128], ex[:, dq:dq + 128], mask)
                        for qql in range(4):
                            qq = 4 * qi + qql
                            if kb <= qq:
                                _mm(nc, accs[qql][:, 0:129],
                                    ex[:, qql * 128:(qql + 1) * 128],
                                    vcat[b][:, kb, h, 0:129],
                                    kb == 0, kb == qq)
                        weave.step()
                    for qql in range(4):
                        rc = sm.tile([128, 1], F32, tag="rc")
                        nc.vector.reciprocal(rc, accs[qql][:, 128:129])
                        an = sm.tile([128, 128], F32R, tag="an")
                        nc.vector.tensor_scalar_mul(
                            out=an, in0=accs[qql][:, 0:128], scalar1=rc)
                        pst = psa.tile([128, 128], F32R, tag="sc", bufs=2)
                        nc.tensor.transpose(pst, an, ident)
                        nc.vector.tensor_copy(stb[b][:, qi, qql, h, :], pst)
                        weave.step()
            weave.drain()

        # ---- schedule ----
        _drain(qkv_quanta(0))
        with tc.tile_pool(name="psa0", bufs=1, space="PSUM") as psa0:
            attention(0, psa0, _Pacer(qkv_quanta(1), total=48, slots=112))
        with tc.tile_pool(name="psa1", bufs=1, space="PSUM") as psa1:
            op0 = outproj_quanta(0, psa1)
            attention(1, psa1, _Pacer(op0, total=128, slots=112))
            _drain(outproj_quanta(1, psa1))

    nc.compile()
    return nc


def _get_nc():
    global _cached_nc
    if _cached_nc is None:
        _cached_nc = build_nc()
    return _cached_nc


def make_in_maps(X, Wq, bq, Wk, bk, Wv, bv, Wo, bo):
    X = np.asarray(X, dtype=np.float32)
    scale = np.float32(1.0 / np.sqrt(DH))
    XT = np.ascontiguousarray(X.transpose(0, 2, 1)).astype(BF16NP)
    mask = (np.arange(128)[None, :] >= np.arange(128)[:, None])
    mask = mask.astype(BF16NP)
    ident = np.eye(128, dtype=np.float32)
    in_maps = []
    for c in range(N_CORES):
        hs = slice(c * DHL, (c + 1) * DHL)
        in_maps.append({
            "xt": XT,
            "wq": np.ascontiguousarray(
                np.asarray(Wq, np.float32)[:, hs] * scale).astype(BF16NP),
            "wk": np.ascontiguousarray(
                np.asarray(Wk, np.float32)[:, hs]).astype(BF16NP),
            "wv": np.ascontiguousarray(
                np.asarray(Wv, np.float32)[:, hs]).astype(BF16NP),
            "bqt": np.ascontiguousarray(
                (np.asarray(bq, np.float32)[hs] * scale).reshape(HL, 128).T),
            "wo": np.ascontiguousarray(
                np.asarray(Wo, np.float32)[hs, :].reshape(HL, 128, D)),
            "maskt": mask,
            "ident": ident,
        })
    return in_maps


def kernel(X, Wq, bq, Wk, bk, Wv, bv, Wo, bo, _trace=False):
    nc = _get_nc()
    in_maps = make_in_maps(X, Wq, bq, Wk, bk, Wv, bv, Wo, bo)
    res = run_bass_kernel_spmd(nc, in_maps, list(range(N_CORES)), trace=_trace)
    acc = res.results[0]["out"].astype(np.float64)
    for c in range(1, N_CORES):
        acc += res.results[c]["out"].astype(np.float64)
    # bv commutes through softmax: sum_k w_k (v_k + bv) = (sum_k w_k v_k) + bv,
    # so the V bias contributes bv @ Wo, folded here with bo. bk shifts every
    # score for a given query by the same constant, so it cancels in softmax.
    acc += np.asarray(bo, np.float64) + (
        np.asarray(bv, np.float64) @ np.asarray(Wo, np.float64))
    out = acc.astype(np.float32)
    if _trace:
        return out, res
    return out


# revision 12
# speedup vs baseline: 1.6062x; 1.0284x over previous
"""Trainium2 Bass kernel for multi-head causal self-attention.

Tensor-parallel over 8 NeuronCores: each core owns 2 of the 16 heads.
Per core (SPMD, identical program, different weight shards):
  - QKV projections for its 2 heads (bf16 data, f32 psum accumulate)
  - causal attention for its 2 heads (scores kept transposed [k, q];
    softmax denominator via a ones-column fused into the PV matmul,
    PV streams N=129 in bf16)
  - output projection partial (f32r), deferred and woven into the next
    phase so the tensor engine never starves behind scalar-engine exps
Phase schedule (tensor-heavy QKV woven into scalar-heavy attention):
  QKV(b0) -> att(b0) + QKV(b1) woven -> att(b1) + outproj(b0) woven
  -> outproj(b1) tail.
Host: shards weights, pre-transposes X to bf16, sums the 8 partials,
adds bo (+ bv@Wo; bk drops out of softmax).

Engine split: tensor = all matmuls; scalar = exp only; vector = psum
copy-outs, bias adds, reciprocals, mask multiplies, ones memset.
"""
import numpy as np
from contextlib import ExitStack

import ml_dtypes
import concourse.bass as bass
import concourse.tile as tile
from concourse import bacc, mybir
from concourse.bass_utils import run_bass_kernel_spmd

# Problem shape (hardcoded per contract)
B, S, D = 2, 2048, 2048
H, DH = 16, 128
N_CORES = 8
HL = H // N_CORES          # heads per core = 2
DHL = HL * DH              # 256
SC = 256                   # s-chunk for projections
NSC = S // SC              # 8 chunks per batch
NKB = S // 128             # 16 key blocks per batch
NQI = S // 512             # 4 q-chunks of 512 per batch
NDC = D // 128             # 16 contraction blocks

F32 = mybir.dt.float32
F32R = mybir.dt.float32r
BF16 = mybir.dt.bfloat16
AF = mybir.ActivationFunctionType
BF16NP = ml_dtypes.bfloat16

_cached_nc = None


def _mm(nc, out, lhsT, rhs, start, stop):
    nc.tensor.matmul(out, lhsT, rhs, start=start, stop=stop)


def _drain(gen):
    if gen is not None:
        for _ in gen:
            pass


class _Pacer:
    """Spread `total` weave quanta over `slots` call sites."""

    def __init__(self, gen, total, slots):
        self.gen = gen
        self.total = total
        self.slots = max(slots, 1)
        self.emitted = 0
        self.slot = 0

    def step(self):
        self.slot += 1
        while (self.gen is not None
               and self.emitted * self.slots < self.slot * self.total):
            try:
                next(self.gen)
                self.emitted += 1
            except StopIteration:
                self.gen = None

    def drain(self):
        _drain(self.gen)
        self.gen = None


def build_nc():
    nc = bacc.Bacc("TRN2", target_bir_lowering=False, debug=False, num_devices=N_CORES)

    # X pre-chunked host-side: xt[p, b, sc, n, s'] = X[b, sc*SC+s', n*128+p]
    xt = nc.dram_tensor("xt", [128, B * NSC, NDC, SC], BF16,
                        kind="ExternalInput").ap()
    # weights pre-arranged host-side to partition-major SBUF layout
    wq = nc.dram_tensor("wq", [128, NDC, DHL], BF16, kind="ExternalInput").ap()
    wk = nc.dram_tensor("wk", [128, NDC, DHL], BF16, kind="ExternalInput").ap()
    wv = nc.dram_tensor("wv", [128, NDC, DHL], BF16, kind="ExternalInput").ap()
    bqt_d = nc.dram_tensor("bqt", [128, HL], F32, kind="ExternalInput").ap()
    wo = nc.dram_tensor("wo", [HL, 128, D], F32R, kind="ExternalInput").ap()
    mask_d = nc.dram_tensor("maskt", [128, 128], BF16, kind="ExternalInput").ap()
    ident_d = nc.dram_tensor("ident", [128, 128], F32R, kind="ExternalInput").ap()
    out = nc.dram_tensor("out", [B, S, D], BF16, kind="ExternalOutput").ap()


    with tile.TileContext(nc) as tc, ExitStack() as ctx:
        pp = ctx.enter_context(tc.tile_pool(name="persist", bufs=1))

        wq_t = pp.tile([128, NDC, DHL], BF16)
        wk_t = pp.tile([128, NDC, DHL], BF16)
        wv_t = pp.tile([128, NDC, DHL], BF16)
        wo_t = pp.tile([128, HL, D], F32R)
        bqt = pp.tile([128, HL], F32)
        mask = pp.tile([128, 128], BF16)
        ident = pp.tile([128, 128], F32R)

        nc.sync.dma_start(out=wq_t, in_=wq)
        nc.sync.dma_start(out=bqt, in_=bqt_d)

        # per-batch persistent activations
        qt = [pp.tile([128, HL, S], BF16, name=f"qt{b}") for b in range(B)]
        kt = [pp.tile([128, HL, S], BF16, name=f"kt{b}") for b in range(B)]
        vcat = [pp.tile([128, NKB, HL, 130], BF16, name=f"vcat{b}")
                for b in range(B)]
        # attn^T blocks awaiting (deferred) output projection
        stb = [pp.tile([128, NQI, 4, HL, 128], F32R, name=f"stb{b}")
               for b in range(B)]

        xp = ctx.enter_context(tc.tile_pool(name="xtp", bufs=3))
        sm = ctx.enter_context(tc.tile_pool(name="smp", bufs=2))
        xpool = ctx.enter_context(tc.tile_pool(name="expool", bufs=2))

        def qkv_quanta(b):
            """Generator: QKV projections for batch b, yielding after each
            16-matmul accumulation group (the weave quantum)."""
            with tc.tile_pool(name=f"psp{b}", bufs=1, space="PSUM") as psp:
                nc.vector.memset(vcat[b][:, :, :, 128:129], 1.0)
                for sc in range(NSC):
                    xt_t = xp.tile([128, NDC, SC], BF16, tag="xt")
                    nc.gpsimd.dma_start(out=xt_t, in_=xt[:, b * NSC + sc])
                    if b == 0 and sc == 0:
                        nc.sync.dma_start(out=wk_t, in_=wk)
                        nc.sync.dma_start(out=wv_t, in_=wv)
                    for h in range(HL):
                        # q and k share one bank: their accumulation groups
                        # are strictly sequential (q closes before k opens)
                        pqk = psp.tile([128, 2, SC], F32, tag="pqk", bufs=1)
                        for dc in range(NDC):
                            _mm(nc, pqk[:, 0, :], wq_t[:, dc, h * 128:(h + 1) * 128],
                                xt_t[:, dc, :], dc == 0, dc == NDC - 1)
                        nc.vector.tensor_scalar_add(
                            out=qt[b][:, h, sc * SC:(sc + 1) * SC],
                            in0=pqk[:, 0, :], scalar1=bqt[:, h:h + 1])
                        yield
                        for dc in range(NDC):
                            _mm(nc, pqk[:, 1, :], wk_t[:, dc, h * 128:(h + 1) * 128],
                                xt_t[:, dc, :], dc == 0, dc == NDC - 1)
                        nc.vector.tensor_copy(
                            kt[b][:, h, sc * SC:(sc + 1) * SC], pqk[:, 1, :])
                        yield
                    psv = psp.tile([128, 2, DHL], F32, tag="pv", bufs=1)
                    for sb in range(SC // 128):
                        kb = sc * (SC // 128) + sb
                        pv = psv[:, sb, :]
                        for dc in range(NDC):
                            _mm(nc, pv, xt_t[:, dc, sb * 128:(sb + 1) * 128],
                                wv_t[:, dc, :], dc == 0, dc == NDC - 1)
                        nc.vector.tensor_copy(
                            vcat[b][:, kb, 0, 0:128], pv[:, 0:128])
                        nc.vector.tensor_copy(
                            vcat[b][:, kb, 1, 0:128], pv[:, 128:256])
                        yield
                if b == 0:
                    nc.sync.dma_start(out=wo_t[:, 0, :], in_=wo[0])
                    nc.sync.dma_start(out=wo_t[:, 1, :], in_=wo[1])
                    nc.sync.dma_start(out=mask, in_=mask_d)
                    nc.sync.dma_start(out=ident, in_=ident_d)

        def outproj_quanta(b, psa, use_scalar=False):
            """Generator: output projection for batch b from stb[b], using
            the po tag of the given PSUM pool. Yields per 512-col chunk.
            Copies rotate across gpsimd/vector (and scalar when idle)."""
            ncopy = 0
            for qi in range(NQI):
                for qql in range(4):
                    qq = 4 * qi + qql
                    for dk in range(D // 512):
                        po = psa.tile([128, 512], F32, tag="po", bufs=2)
                        _mm(nc, po, stb[b][:, qi, qql, 0, :],
                            wo_t[:, 0, dk * 512:(dk + 1) * 512], True, False)
                        _mm(nc, po, stb[b][:, qi, qql, 1, :],
                            wo_t[:, 1, dk * 512:(dk + 1) * 512], False, True)
                        ot = sm.tile([128, 512], BF16, tag="ot", bufs=6)
                        ncopy += 1
                        nc.vector.tensor_copy(ot, po)  # BISECT: vector only
                        nc.sync.dma_start(
                            out=out[b, qq * 128:(qq + 1) * 128,
                                    dk * 512:(dk + 1) * 512],
                            in_=ot)
                        yield

        def attention(b, psa, weave):
            """Causal attention for batch b; calls weave.step() at each
            opportunity to inject foreign tensor-engine work."""
            for qi in range(NQI):
                for h in range(HL):
                    accs = [psa.tile([128, 256], F32, tag=f"acc{i}",
                                     bufs=1, name=f"acc{i}")
                            for i in range(4)]
                    nkb = 4 * qi + 4
                    pss_t = {}

                    def score(kb):
                        dq = max(0, (kb - 4 * qi)) * 128
                        pss = psa.tile([128, 512], F32, tag="sc", bufs=2)
                        _mm(nc, pss[:, dq:512],
                            kt[b][:, h, kb * 128:(kb + 1) * 128],
                            qt[b][:, h, qi * 512 + dq:(qi + 1) * 512],
                            True, True)
                        pss_t[kb] = pss

                    score(0)
                    for kb in range(nkb):
                        dq = max(0, (kb - 4 * qi)) * 128
                        pss = pss_t.pop(kb)
                        ex = xpool.tile([128, 512], BF16, tag="ex", bufs=6)
                        nc.scalar.activation(
                            out=ex[:, dq:512], in_=pss[:, dq:512],
                            func=AF.Exp)
                        if kb + 1 < nkb:
                            score(kb + 1)
                        if kb >= 4 * qi:
                            nc.vector.tensor_mul(
                                ex[:, dq:dq + 128], ex[:, dq:dq + 128], mask)
                        for qql in range(4):
                            qq = 4 * qi + qql
                            if kb <= qq:
                                _mm(nc, accs[qql][:, 0:129],
                                    ex[:, qql * 128:(qql + 1) * 128],
                                    vcat[b][:, kb, h, 0:129],
                                    kb == 0, kb == qq)
                        weave.step()
                    for qql in range(4):
                        rc = sm.tile([128, 1], F32, tag="rc")
                        nc.vector.reciprocal(rc, accs[qql][:, 128:129])
                        an = sm.tile([128, 128], F32R, tag="an")
                        nc.vector.tensor_scalar_mul(
                            out=an, in0=accs[qql][:, 0:128], scalar1=rc)
                        pst = psa.tile([128, 128], F32R, tag="sc", bufs=2)
                        nc.tensor.transpose(pst, an, ident)
                        nc.vector.tensor_copy(stb[b][:, qi, qql, h, :], pst)
                        weave.step()
            weave.drain()

        # ---- schedule ----
        _drain(qkv_quanta(0))
        with tc.tile_pool(name="psa0", bufs=1, space="PSUM") as psa0:
            attention(0, psa0, _Pacer(qkv_quanta(1), total=48, slots=112))
        with tc.tile_pool(name="psa1", bufs=1, space="PSUM") as psa1:
            op0 = outproj_quanta(0, psa1)
            attention(1, psa1, _Pacer(op0, total=128, slots=112))
            _drain(outproj_quanta(1, psa1, use_scalar=True))

    nc.compile()
    return nc


def _get_nc():
    global _cached_nc
    if _cached_nc is None:
        _cached_nc = build_nc()
    return _cached_nc


def make_in_maps(X, Wq, bq, Wk, bk, Wv, bv, Wo, bo):
    X = np.asarray(X, dtype=np.float32)
    scale = np.float32(1.0 / np.sqrt(DH))
    # [128, B, NSC, NDC, SC]: xt[p, b, sc, n, s'] = X[b, sc*SC+s', n*128+p]
    XT = np.ascontiguousarray(
        X.reshape(B, NSC, SC, NDC, 128).transpose(4, 0, 1, 3, 2)
    ).reshape(128, B * NSC, NDC, SC).astype(BF16NP)

    def pmaj(w):  # [D, DHL] -> [128, NDC, DHL] partition-major
        return np.ascontiguousarray(
            w.reshape(NDC, 128, DHL).transpose(1, 0, 2))
    mask = (np.arange(128)[None, :] >= np.arange(128)[:, None])
    mask = mask.astype(BF16NP)
    ident = np.eye(128, dtype=np.float32)
    in_maps = []
    for c in range(N_CORES):
        hs = slice(c * DHL, (c + 1) * DHL)
        in_maps.append({
            "xt": XT,
            "wq": pmaj(np.asarray(Wq, np.float32)[:, hs] * scale).astype(BF16NP),
            "wk": pmaj(np.asarray(Wk, np.float32)[:, hs]).astype(BF16NP),
            "wv": pmaj(np.asarray(Wv, np.float32)[:, hs]).astype(BF16NP),
            "bqt": np.ascontiguousarray(
                (np.asarray(bq, np.float32)[hs] * scale).reshape(HL, 128).T),
            "wo": np.ascontiguousarray(
                np.asarray(Wo, np.float32)[hs, :].reshape(HL, 128, D)),
            "maskt": mask,
            "ident": ident,
        })
    return in_maps


def kernel(X, Wq, bq, Wk, bk, Wv, bv, Wo, bo, _trace=False):
    nc = _get_nc()
    in_maps = make_in_maps(X, Wq, bq, Wk, bk, Wv, bv, Wo, bo)
    res = run_bass_kernel_spmd(nc, in_maps, list(range(N_CORES)), trace=_trace)
    acc = res.results[0]["out"].astype(np.float64)
    for c in range(1, N_CORES):
        acc += res.results[c]["out"].astype(np.float64)
    # bv commutes through softmax: sum_k w_k (v_k + bv) = (sum_k w_k v_k) + bv,
    # so the V bias contributes bv @ Wo, folded here with bo. bk shifts every
    # score for a given query by the same constant, so it cancels in softmax.
    acc += np.asarray(bo, np.float64) + (
        np.asarray(bv, np.float64) @ np.asarray(Wo, np.float64))
    out = acc.astype(np.float32)
    if _trace:
        return out, res
    return out


# revision 13
# speedup vs baseline: 1.6581x; 1.0323x over previous
"""Trainium2 Bass kernel for multi-head causal self-attention.

Tensor-parallel over 8 NeuronCores: each core owns 2 of the 16 heads.
Per core (SPMD, identical program, different weight shards):
  - QKV projections for its 2 heads (bf16 data, f32 psum accumulate)
  - causal attention for its 2 heads (scores kept transposed [k, q];
    softmax denominator via a ones-column fused into the PV matmul,
    PV streams N=129 in bf16)
  - output projection partial (f32r), deferred and woven into the next
    phase so the tensor engine never starves behind scalar-engine exps
Phase schedule (tensor-heavy QKV woven into scalar-heavy attention):
  QKV(b0) -> att(b0) + QKV(b1) woven -> att(b1) + outproj(b0) woven
  -> outproj(b1) tail.
Host: shards weights, pre-transposes X to bf16, sums the 8 partials,
adds bo (+ bv@Wo; bk drops out of softmax).

Engine split: tensor = all matmuls; scalar = exp only; vector = psum
copy-outs, bias adds, reciprocals, mask multiplies, ones memset.
"""
import numpy as np
from contextlib import ExitStack

import ml_dtypes
import concourse.bass as bass
import concourse.tile as tile
from concourse import bacc, mybir
from concourse.bass_utils import run_bass_kernel_spmd

# Problem shape (hardcoded per contract)
B, S, D = 2, 2048, 2048
H, DH = 16, 128
N_CORES = 8
HL = H // N_CORES          # heads per core = 2
DHL = HL * DH              # 256
SC = 256                   # s-chunk for projections
NSC = S // SC              # 8 chunks per batch
NKB = S // 128             # 16 key blocks per batch
NQI = S // 512             # 4 q-chunks of 512 per batch
NDC = D // 128             # 16 contraction blocks

F32 = mybir.dt.float32
F32R = mybir.dt.float32r
BF16 = mybir.dt.bfloat16
AF = mybir.ActivationFunctionType
BF16NP = ml_dtypes.bfloat16

_cached_nc = None


def _mm(nc, out, lhsT, rhs, start, stop):
    nc.tensor.matmul(out, lhsT, rhs, start=start, stop=stop)


def _drain(gen):
    if gen is not None:
        for _ in gen:
            pass


class _Pacer:
    """Spread `total` weave quanta over `slots` call sites."""

    def __init__(self, gen, total, slots):
        self.gen = gen
        self.total = total
        self.slots = max(slots, 1)
        self.emitted = 0
        self.slot = 0

    def step(self):
        self.slot += 1
        while (self.gen is not None
               and self.emitted * self.slots < self.slot * self.total):
            try:
                next(self.gen)
                self.emitted += 1
            except StopIteration:
                self.gen = None

    def drain(self):
        _drain(self.gen)
        self.gen = None


def build_nc():
    nc = bacc.Bacc("TRN2", target_bir_lowering=False, debug=False, num_devices=N_CORES)

    # X pre-chunked host-side: xt[p, b, sc, n, s'] = X[b, sc*SC+s', n*128+p]
    xt = nc.dram_tensor("xt", [128, B * NSC, NDC, SC], BF16,
                        kind="ExternalInput").ap()
    # weights pre-arranged host-side to partition-major SBUF layout
    wq = nc.dram_tensor("wq", [128, NDC, DHL], BF16, kind="ExternalInput").ap()
    wk = nc.dram_tensor("wk", [128, NDC, DHL], BF16, kind="ExternalInput").ap()
    wv = nc.dram_tensor("wv", [128, NDC, DHL], BF16, kind="ExternalInput").ap()
    bqt_d = nc.dram_tensor("bqt", [128, HL], F32, kind="ExternalInput").ap()
    wo = nc.dram_tensor("wo", [HL, 128, D], F32R, kind="ExternalInput").ap()
    mask_d = nc.dram_tensor("maskt", [128, 128], BF16, kind="ExternalInput").ap()
    ident_d = nc.dram_tensor("ident", [128, 128], F32R, kind="ExternalInput").ap()
    out = nc.dram_tensor("out", [B, S, D], BF16, kind="ExternalOutput").ap()


    with tile.TileContext(nc) as tc, ExitStack() as ctx:
        pp = ctx.enter_context(tc.tile_pool(name="persist", bufs=1))

        wq_t = pp.tile([128, NDC, DHL], BF16)
        wk_t = pp.tile([128, NDC, DHL], BF16)
        wv_t = pp.tile([128, NDC, DHL], BF16)
        wo_t = pp.tile([128, HL, D], F32R)
        bqt = pp.tile([128, HL], F32)
        mask = pp.tile([128, 128], BF16)
        ident = pp.tile([128, 128], F32R)

        nc.sync.dma_start(out=wq_t, in_=wq)
        nc.sync.dma_start(out=bqt, in_=bqt_d)

        # per-batch persistent activations
        qt = [pp.tile([128, HL, S], BF16, name=f"qt{b}") for b in range(B)]
        kt = [pp.tile([128, HL, S], BF16, name=f"kt{b}") for b in range(B)]
        vcat = [pp.tile([128, NKB, HL, 130], BF16, name=f"vcat{b}")
                for b in range(B)]
        # attn^T blocks awaiting (deferred) output projection
        stb = [pp.tile([128, NQI, 4, HL, 128], F32R, name=f"stb{b}")
               for b in range(B)]

        xp = ctx.enter_context(tc.tile_pool(name="xtp", bufs=3))
        sm = ctx.enter_context(tc.tile_pool(name="smp", bufs=2))
        xpool = ctx.enter_context(tc.tile_pool(name="expool", bufs=2))

        def qkv_quanta(b):
            """Generator: QKV projections for batch b, yielding after each
            16-matmul accumulation group (the weave quantum)."""
            with tc.tile_pool(name=f"psp{b}", bufs=1, space="PSUM") as psp:
                nc.vector.memset(vcat[b][:, :, :, 128:129], 1.0)
                for sc in range(NSC):
                    xt_t = xp.tile([128, NDC, SC], BF16, tag="xt")
                    nc.gpsimd.dma_start(out=xt_t, in_=xt[:, b * NSC + sc])
                    if b == 0 and sc == 0:
                        nc.sync.dma_start(out=wk_t, in_=wk)
                        nc.sync.dma_start(out=wv_t, in_=wv)
                    for h in range(HL):
                        # q and k share one bank: their accumulation groups
                        # are strictly sequential (q closes before k opens)
                        pqk = psp.tile([128, 2, SC], F32, tag="pqk", bufs=1)
                        for dc in range(NDC):
                            _mm(nc, pqk[:, 0, :], wq_t[:, dc, h * 128:(h + 1) * 128],
                                xt_t[:, dc, :], dc == 0, dc == NDC - 1)
                        nc.vector.tensor_scalar_add(
                            out=qt[b][:, h, sc * SC:(sc + 1) * SC],
                            in0=pqk[:, 0, :], scalar1=bqt[:, h:h + 1])
                        yield
                        for dc in range(NDC):
                            _mm(nc, pqk[:, 1, :], wk_t[:, dc, h * 128:(h + 1) * 128],
                                xt_t[:, dc, :], dc == 0, dc == NDC - 1)
                        nc.vector.tensor_copy(
                            kt[b][:, h, sc * SC:(sc + 1) * SC], pqk[:, 1, :])
                        yield
                    psv = psp.tile([128, 2, DHL], F32, tag="pv", bufs=1)
                    for sb in range(SC // 128):
                        kb = sc * (SC // 128) + sb
                        pv = psv[:, sb, :]
                        for dc in range(NDC):
                            _mm(nc, pv, xt_t[:, dc, sb * 128:(sb + 1) * 128],
                                wv_t[:, dc, :], dc == 0, dc == NDC - 1)
                        nc.vector.tensor_copy(
                            vcat[b][:, kb, 0, 0:128], pv[:, 0:128])
                        nc.vector.tensor_copy(
                            vcat[b][:, kb, 1, 0:128], pv[:, 128:256])
                        yield
                if b == 0:
                    nc.sync.dma_start(out=wo_t[:, 0, :], in_=wo[0])
                    nc.sync.dma_start(out=wo_t[:, 1, :], in_=wo[1])
                    nc.sync.dma_start(out=mask, in_=mask_d)
                    nc.sync.dma_start(out=ident, in_=ident_d)

        def outproj_quanta(b, psa, use_scalar=False):
            """Generator: output projection for batch b from stb[b], using
            the po tag of the given PSUM pool. Yields per 512-col chunk.
            Copies rotate across gpsimd/vector (and scalar when idle)."""
            ncopy = 0
            for qi in range(NQI):
                for qql in range(4):
                    qq = 4 * qi + qql
                    for dk in range(D // 512):
                        po = psa.tile([128, 512], F32, tag="po", bufs=2)
                        _mm(nc, po, stb[b][:, qi, qql, 0, :],
                            wo_t[:, 0, dk * 512:(dk + 1) * 512], True, False)
                        _mm(nc, po, stb[b][:, qi, qql, 1, :],
                            wo_t[:, 1, dk * 512:(dk + 1) * 512], False, True)
                        ot = sm.tile([128, 512], BF16, tag="ot", bufs=6)
                        nmod = ncopy % 2
                        ncopy += 1
                        if nmod == 0:
                            nc.scalar.activation(out=ot, in_=po, func=AF.Copy)
                        else:
                            nc.vector.tensor_copy(ot, po)
                        nc.sync.dma_start(
                            out=out[b, qq * 128:(qq + 1) * 128,
                                    dk * 512:(dk + 1) * 512],
                            in_=ot)
                        yield

        def attention(b, psa, weave):
            """Causal attention for batch b; calls weave.step() at each
            opportunity to inject foreign tensor-engine work."""
            for qi in range(NQI):
                for h in range(HL):
                    accs = [psa.tile([128, 256], F32, tag=f"acc{i}",
                                     bufs=1, name=f"acc{i}")
                            for i in range(4)]
                    nkb = 4 * qi + 4
                    pss_t = {}

                    def score(kb):
                        dq = max(0, (kb - 4 * qi)) * 128
                        pss = psa.tile([128, 512], F32, tag="sc", bufs=2)
                        _mm(nc, pss[:, dq:512],
                            kt[b][:, h, kb * 128:(kb + 1) * 128],
                            qt[b][:, h, qi * 512 + dq:(qi + 1) * 512],
                            True, True)
                        pss_t[kb] = pss

                    score(0)
                    for kb in range(nkb):
                        dq = max(0, (kb - 4 * qi)) * 128
                        pss = pss_t.pop(kb)
                        ex = xpool.tile([128, 512], BF16, tag="ex", bufs=6)
                        nc.scalar.activation(
                            out=ex[:, dq:512], in_=pss[:, dq:512],
                            func=AF.Exp)
                        if kb + 1 < nkb:
                            score(kb + 1)
                        if kb >= 4 * qi:
                            nc.vector.tensor_mul(
                                ex[:, dq:dq + 128], ex[:, dq:dq + 128], mask)
                        for qql in range(4):
                            qq = 4 * qi + qql
                            if kb <= qq:
                                _mm(nc, accs[qql][:, 0:129],
                                    ex[:, qql * 128:(qql + 1) * 128],
                                    vcat[b][:, kb, h, 0:129],
                                    kb == 0, kb == qq)
                        weave.step()
                    for qql in range(4):
                        rc = sm.tile([128, 1], F32, tag="rc")
                        nc.vector.reciprocal(rc, accs[qql][:, 128:129])
                        an = sm.tile([128, 128], F32R, tag="an")
                        nc.vector.tensor_scalar_mul(
                            out=an, in0=accs[qql][:, 0:128], scalar1=rc)
                        pst = psa.tile([128, 128], F32R, tag="sc", bufs=2)
                        nc.tensor.transpose(pst, an, ident)
                        nc.vector.tensor_copy(stb[b][:, qi, qql, h, :], pst)
                        weave.step()
            weave.drain()

        # ---- schedule ----
        _drain(qkv_quanta(0))
        with tc.tile_pool(name="psa0", bufs=1, space="PSUM") as psa0:
            attention(0, psa0, _Pacer(qkv_quanta(1), total=48, slots=112))
        with tc.tile_pool(name="psa1", bufs=1, space="PSUM") as psa1:
            op0 = outproj_quanta(0, psa1)
            attention(1, psa1, _Pacer(op0, total=128, slots=112))
            _drain(outproj_quanta(1, psa1, use_scalar=True))

    nc.compile()
    return nc


def _get_nc():
    global _cached_nc
    if _cached_nc is None:
        _cached_nc = build_nc()
    return _cached_nc


def make_in_maps(X, Wq, bq, Wk, bk, Wv, bv, Wo, bo):
    X = np.asarray(X, dtype=np.float32)
    scale = np.float32(1.0 / np.sqrt(DH))
    # [128, B, NSC, NDC, SC]: xt[p, b, sc, n, s'] = X[b, sc*SC+s', n*128+p]
    XT = np.ascontiguousarray(
        X.reshape(B, NSC, SC, NDC, 128).transpose(4, 0, 1, 3, 2)
    ).reshape(128, B * NSC, NDC, SC).astype(BF16NP)

    def pmaj(w):  # [D, DHL] -> [128, NDC, DHL] partition-major
        return np.ascontiguousarray(
            w.reshape(NDC, 128, DHL).transpose(1, 0, 2))
    mask = (np.arange(128)[None, :] >= np.arange(128)[:, None])
    mask = mask.astype(BF16NP)
    ident = np.eye(128, dtype=np.float32)
    in_maps = []
    for c in range(N_CORES):
        hs = slice(c * DHL, (c + 1) * DHL)
        in_maps.append({
            "xt": XT,
            "wq": pmaj(np.asarray(Wq, np.float32)[:, hs] * scale).astype(BF16NP),
            "wk": pmaj(np.asarray(Wk, np.float32)[:, hs]).astype(BF16NP),
            "wv": pmaj(np.asarray(Wv, np.float32)[:, hs]).astype(BF16NP),
            "bqt": np.ascontiguousarray(
                (np.asarray(bq, np.float32)[hs] * scale).reshape(HL, 128).T),
            "wo": np.ascontiguousarray(
                np.asarray(Wo, np.float32)[hs, :].reshape(HL, 128, D)),
            "maskt": mask,
            "ident": ident,
        })
    return in_maps


def kernel(X, Wq, bq, Wk, bk, Wv, bv, Wo, bo, _trace=False):
    nc = _get_nc()
    in_maps = make_in_maps(X, Wq, bq, Wk, bk, Wv, bv, Wo, bo)
    res = run_bass_kernel_spmd(nc, in_maps, list(range(N_CORES)), trace=_trace)
    acc = res.results[0]["out"].astype(np.float64)
    for c in range(1, N_CORES):
        acc += res.results[c]["out"].astype(np.float64)
    # bv commutes through softmax: sum_k w_k (v_k + bv) = (sum_k w_k v_k) + bv,
    # so the V bias contributes bv @ Wo, folded here with bo. bk shifts every
    # score for a given query by the same constant, so it cancels in softmax.
    acc += np.asarray(bo, np.float64) + (
        np.asarray(bv, np.float64) @ np.asarray(Wo, np.float64))
    out = acc.astype(np.float32)
    if _trace:
        return out, res
    return out
